# revision 1
# baseline (speedup 1.0000x reference)
"""Trainium2 Bass kernel for a causal multi-head-attention block (v2).

Reference computation (B=4, S=2048, D=1024, H=16, DK=64), torch Linear
convention (x @ W.T + b):
    Q = q @ wq.T + bq ; K = k @ wk.T + bk ; V = v @ wv.T + bv
    per head: attn = softmax(mask(Q K^T / sqrt(DK))) ; x = attn @ V
    out = concat_heads(x) @ wo.T + bo
Sharding: 8 cores = data-parallel over batch (4) x tensor-parallel over
heads (2); host sums the two TP partial outputs per batch and adds bo.

v2 structural changes over the two-stage baseline:
  - The kernel is scheduled ScalarE-forward: exp is the scarce resource
    (160 instructions, ~0.9-2us each), so attention rows run in order
    [2, 3, 1, 0].  Row 2 unlocks after only K0-2 + Q2 are projected
    (~27us), and from there ScalarE runs exp continuously: each row's
    remaining prerequisites (K3/Q3, then Q1, Q0) are projected by the
    PE inside the previous row's exp-bound window, and the output
    projections + leftover V chunks serve as PE filler work.
  - PSUM: scores 2x[128,2,QC] (4 banks) + attn@V accumulator (2 banks)
    + one shared 2-slot pool for projection/output-projection [128,QC]
    tiles (2 banks) = 8 banks exactly.
  - attn@V trails scores/exp by PEND=4 k-tiles (HW-measured optimum;
    deeper trails let the PE run ahead of ScalarE so exp never starves).
  - The diagonal-block mask multiply keeps the stride-0 head-broadcast
    AP: HW-measured 576 ns vs 980 ns for a dense fp16 operand.

Measured on HW (differential reps timing, 8 cores): 369-373 us/core vs
446 us for the two-stage baseline; rel err 7.075e-04 (fp16 matmuls,
fp32 PSUM/softmax denominators).  Rejected variants (all HW-measured
slower): zero-padded 128-row score stationaries (391), batched attn@V
pops (384), normalize-from-PSUM without the ub2 evacuation (426+, the
evacuation frees the po2 banks for the next head pair), fp16 outT
(385), mask muls on GpSimd (479), row order 1320 (397), outproj
emitted after the next row (415).

On-chip dataflow is fully transposed (no on-chip transposes):
  - QT/KT computed as [dq, s] tiles (head pairs on partitions)
  - scoresT[k, q] = KT.T-tile @ QT (two heads row-tiled on the PE array)
  - exp directly out of PSUM on ScalarE (no max subtraction: scores are
    O(6) here, exp is safely bounded in fp32)
  - attn@V with stationary V_aug [k, 65] whose 65th column of ones yields
    the softmax denominator for free
  - output projection consumes the [d, q] layout directly, emits outT
"""

import sys

sys.path.insert(0, "/opt/trn_rl_repo")

import numpy as np

import concourse.bass as bass
import concourse.mybir as mybir
import concourse.tile as tile
from concourse import bacc
from concourse.bass_utils import run_bass_kernel_spmd

B, S, D, H, DK = 4, 2048, 1024, 16, 64
NCORES = 8
TP = 2  # tensor-parallel ways (head split)
HL = H // TP  # 8 local heads
DL = D // TP  # 512 local projection dims
QC = 512  # q-chunk (moving free dim)
NS = S // QC  # 4 q-chunks
NI = D // 128  # 8 contraction tiles for projections
NP = DL // 128  # 4 head pairs per core
NT = S // 128  # 16 k-tiles
F32 = mybir.dt.float32
F16 = mybir.dt.float16
ACTF = mybir.ActivationFunctionType

import os
import ml_dtypes

MDT = F16
NP_MDT = np.float16
PEND = int(os.environ.get("KERNEL_PEND", "4"))  # attn@V trail depth (k-tiles)
PAIRKT = int(os.environ.get("KERNEL_PAIRKT", "0"))  # batch attn@V pops in pairs
MASKPOOL = int(os.environ.get("KERNEL_MASKPOOL", "0"))  # mask muls on GpSimd
ROWORD = os.environ.get("KERNEL_ROWORD", "2310")  # attention row order


def build(reps: int = 1):
    """Build + compile the per-core Bass program (same program on all cores)."""
    nc = bacc.Bacc("TRN2", target_bir_lowering=False, num_devices=NCORES)

    qT_d = nc.declare_dram_parameter("qT", [D, S], MDT, isOutput=False)
    kT_d = nc.declare_dram_parameter("kT", [D, S], MDT, isOutput=False)
    vT_d = nc.declare_dram_parameter("vT", [D, S], MDT, isOutput=False)
    wqT_d = nc.declare_dram_parameter("wqT", [D, DL], MDT, isOutput=False)
    wkT_d = nc.declare_dram_parameter("wkT", [D, DL], MDT, isOutput=False)
    wvT_d = nc.declare_dram_parameter("wvT", [D, DL], MDT, isOutput=False)
    woT_d = nc.declare_dram_parameter("woT", [DL, D], MDT, isOutput=False)
    bq_d = nc.declare_dram_parameter("bq_r", [128, NP], F32, isOutput=False)
    bk_d = nc.declare_dram_parameter("bk_r", [128, NP], F32, isOutput=False)
    bv_d = nc.declare_dram_parameter("bv_r", [128, HL, DK], F32, isOutput=False)
    mk_d = nc.declare_dram_parameter("masks", [4, 128, QC], MDT, isOutput=False)
    ones_d = nc.declare_dram_parameter("ones_r", [128, HL], MDT, isOutput=False)
    outT_d = nc.declare_dram_parameter("outT", [D, S], F32, isOutput=True)

    with tile.TileContext(nc) as tc:
        _emit(nc, tc, reps, qT_d, kT_d, vT_d, wqT_d, wkT_d, wvT_d, woT_d,
              bq_d, bk_d, bv_d, mk_d, ones_d, outT_d)
    nc.compile()
    return nc


def _emit(nc, tc, reps, *args):
    def body():
        _emit_once(nc, tc, *args)

    if reps == 1:
        body()
    else:
        with tc.For_i(0, reps, 1):
            body()


def _emit_once(nc, tc, qT_d, kT_d, vT_d, wqT_d, wkT_d, wvT_d, woT_d,
               bq_d, bk_d, bv_d, mk_d, ones_d, outT_d):
    from contextlib import ExitStack

    qT = qT_d[:, :].rearrange("(i p) s -> p i s", p=128)
    kT = kT_d[:, :].rearrange("(i p) s -> p i s", p=128)
    vT = vT_d[:, :].rearrange("(i p) s -> p i s", p=128)
    wT_r = {
        "q": wqT_d[:, :].rearrange("(i p) m -> p i m", p=128),
        "k": wkT_d[:, :].rearrange("(i p) m -> p i m", p=128),
        "v": wvT_d[:, :].rearrange("(i p) m -> p i m", p=128),
    }
    xT_r = {"q": qT, "k": kT, "v": vT}

    with ExitStack() as stack:
        persist = stack.enter_context(tc.tile_pool(name="persist", bufs=1))
        xpool = stack.enter_context(tc.tile_pool(name="xpool", bufs=3))
        epool = stack.enter_context(tc.tile_pool(name="epool", bufs=10))
        apool = stack.enter_context(tc.tile_pool(name="apool", bufs=8))
        rpool = stack.enter_context(tc.tile_pool(name="rpool", bufs=4))
        opool = stack.enter_context(tc.tile_pool(name="opool", bufs=3))
        ps_s = stack.enter_context(tc.tile_pool(name="ps_s", bufs=2, space="PSUM"))
        ps_o = stack.enter_context(tc.tile_pool(name="ps_o", bufs=1, space="PSUM"))
        ps_g = stack.enter_context(tc.tile_pool(name="ps_g", bufs=2, space="PSUM"))

        # Persistent activations: QT/KT [128(=pair dims), NP, S], V_aug.
        QT_sb = persist.tile([128, NP, S], MDT)
        KT_sb = persist.tile([128, NP, S], MDT)
        # V_aug: per k-tile, 8 heads x (64 V cols + 1 ones col)
        V_sb = persist.tile([128, NT, HL * (DK + 1)], MDT)
        v_view = V_sb.rearrange("p t (h j) -> p t h j", j=DK + 1)
        bq_sb = persist.tile([128, NP], F32)
        bk_sb = persist.tile([128, NP], F32)
        bv_sb = persist.tile([128, HL, DK], F32)
        ones_sb = persist.tile([128, HL], MDT)
        mask_sb = persist.tile([128, 4, QC], MDT)
        wo_sb = persist.tile([128, NP, D], MDT)
        wq_sb = persist.tile([128, NI, DL], MDT)
        wk_sb = persist.tile([128, NI, DL], MDT)
        wv_sb = persist.tile([128, NI, DL], MDT)
        w_sb = {"q": wq_sb, "k": wk_sb, "v": wv_sb}

        nc.sync.dma_start(out=bq_sb, in_=bq_d[:, :])
        nc.sync.dma_start(out=bk_sb, in_=bk_d[:, :])
        nc.sync.dma_start(out=bv_sb, in_=bv_d[:, :, :])
        nc.sync.dma_start(out=ones_sb, in_=ones_d[:, :])
        nc.sync.dma_start(out=mask_sb, in_=mk_d[:, :, :].rearrange("d p q -> p d q"))
        # ones columns of V_aug (written once; disjoint from the V writes)
        for t in range(NT):
            nc.gpsimd.tensor_copy(v_view[:, t, :, DK], ones_sb)

        w_loaded = set()

        def load_chunk(which, sc):
            """x-chunk DMA, interleaved per-i with the weight DMAs on the
            first use of each weight so the first matmuls start early."""
            ssl = slice(sc * QC, (sc + 1) * QC)
            x = xpool.tile([128, NI, QC], MDT, tag="xq")
            first = which not in w_loaded
            w_loaded.add(which)
            for it in range(NI):
                nc.sync.dma_start(out=x[:, it, :], in_=xT_r[which][:, it, ssl])
                if first:
                    nc.sync.dma_start(
                        out=w_sb[which][:, it, :], in_=wT_r[which][:, it, :]
                    )
            return x

        def proj_qk(which, dst, bias, sc):
            ssl = slice(sc * QC, (sc + 1) * QC)
            x = load_chunk(which, sc)
            for hp in range(NP):
                psum = ps_g.tile([128, QC], F32, tag="pg")
                for it in range(NI):
                    nc.tensor.matmul(
                        psum,
                        w_sb[which][:, it, hp * 128 : (hp + 1) * 128],
                        x[:, it, :],
                        start=(it == 0),
                        stop=(it == NI - 1),
                    )
                nc.vector.tensor_scalar_add(
                    dst[:, hp, ssl], psum, bias[:, hp : hp + 1]
                )

        def proj_v(sc):
            x = load_chunk("v", sc)
            for st4 in range(4):
                st = 4 * sc + st4
                psum = ps_g.tile([128, QC], F32, tag="pg")
                for it in range(NI):
                    nc.tensor.matmul(
                        psum,
                        x[:, it, st4 * 128 : (st4 + 1) * 128],
                        wv_sb[:, it, :],
                        start=(it == 0),
                        stop=(it == NI - 1),
                    )
                # scatter head columns into the 65-stride V_aug layout,
                # adding the bias in the same pass
                nc.vector.tensor_add(
                    v_view[:, st, :, 0:DK],
                    psum.rearrange("p (h d) -> p h d", d=DK),
                    bv_sb,
                )

        def attn_row(j, fillers=None):
            """Attention row j (q-chunk j over k-tiles 0..4(j+1)-1);
            returns the normalized attn tiles for the output projection.
            fillers: {kt: callback} emitted inside hp 0's k loop — PE work
            (e.g. a V-chunk projection) that fills the exp-bound gaps but
            must not precede the row's scores in priority order."""
            nkt = 4 * (j + 1)
            attn_tiles = []
            for hp in range(NP):
                po2 = ps_o.tile([DK + 1, 2, QC], F32, tag="po")
                # software pipeline: attn@V trails scores/exp by two k-tiles
                # so the PE never waits on the just-issued exp
                pend = []

                def attnv(kt, e0, e1, off):
                    nc.tensor.matmul(
                        po2[:, 0, off:],
                        v_view[:, kt, 2 * hp, :],
                        e0,
                        start=(kt == 0),
                        stop=(kt == nkt - 1),
                    )
                    nc.tensor.matmul(
                        po2[:, 1, off:],
                        v_view[:, kt, 2 * hp + 1, :],
                        e1,
                        start=(kt == 0),
                        stop=(kt == nkt - 1),
                    )

                for kt in range(nkt):
                    ksl = slice(kt * 128, (kt + 1) * 128)
                    dd = kt - 4 * j
                    # diagonal blocks only have live queries at q >= 128*dd
                    off = 128 * dd if dd > 0 else 0
                    jssl = slice(j * QC + off, (j + 1) * QC)
                    # both heads' score blocks in one 2-bank PSUM tile so
                    # a single [128, 2, N'] exp covers the pair
                    ps2 = ps_s.tile([128, 2, QC], F32, tag="ps")
                    nc.tensor.matmul(
                        ps2[:, 0, off:],
                        KT_sb[0:64, hp, ksl],
                        QT_sb[0:64, hp, jssl],
                        start=True,
                        stop=True,
                    )
                    nc.tensor.matmul(
                        ps2[:, 1, off:],
                        KT_sb[64:128, hp, ksl],
                        QT_sb[64:128, hp, jssl],
                        start=True,
                        stop=True,
                        tile_position=(64, 0),
                    )
                    ex2 = epool.tile([128, 2, QC], MDT, tag="ex")
                    nc.scalar.activation(
                        ex2[:, :, off:], ps2[:, :, off:], ACTF.Exp, scale=1.0 / 8.0
                    )
                    if dd >= 0:  # diagonal block: zero the k > q entries
                        # stride-0 head-broadcast AP: measured FASTER on HW
                        # than a dense fp16 operand (576 vs 980 ns)
                        m = mask_sb[:, dd, off:]
                        mb = bass.AP(
                            tensor=m.tensor, offset=m.offset,
                            ap=[list(m.ap[0]), [0, 2], list(m.ap[1])],
                        )
                        eng = nc.gpsimd if MASKPOOL else nc.vector
                        eng.tensor_mul(ex2[:, :, off:], ex2[:, :, off:], mb)
                    pend.append((kt, ex2[:, 0, off:], ex2[:, 1, off:], off))
                    if PAIRKT:
                        # drain two trailing k-tiles at once on odd steps so
                        # the PE sees [scores x2 ... attnv x2] batches (half
                        # the 64/128-row array mode switches)
                        if kt % 2 == 1:
                            while len(pend) > PEND:
                                attnv(*pend.pop(0))
                    elif len(pend) > PEND:
                        attnv(*pend.pop(0))
                    if hp == 0 and fillers and kt in fillers:
                        fillers[kt]()
                for p_ in pend:
                    attnv(*p_)
                # evacuate both accumulators in one op so the ps_o banks
                # free for the next pair; normalize from SBUF
                ub2 = rpool.tile([DK + 1, 2, QC], F32, tag="ub")
                nc.vector.tensor_copy(ub2, po2)
                # normalize: row DK of ub2 is the softmax denominator
                rec = rpool.tile([1, 2, QC], F32, tag="rec")
                nc.vector.reciprocal(rec, ub2[DK : DK + 1, :, :])
                rb = rpool.tile([64, 2, QC], F32, tag="rb")
                nc.gpsimd.partition_broadcast(rb, rec)
                attn = apool.tile([128, QC], MDT, tag="attn")
                nc.vector.tensor_mul(attn[0:64, :], ub2[0:64, 0, :], rb[:, 0, :])
                # h1 written straight to partitions 64..127 (DVE operands
                # may sit at different base partitions)
                nc.vector.tensor_mul(attn[64:128, :], ub2[0:64, 1, :], rb[:, 1, :])
                attn_tiles.append(attn)
            return attn_tiles

        def outproj_row(j, attn_tiles):
            jsl = slice(j * QC, (j + 1) * QC)
            for et in range(NI):
                pc = ps_g.tile([128, QC], F32, tag="pg")
                for hp in range(NP):
                    nc.tensor.matmul(
                        pc,
                        wo_sb[:, hp, et * 128 : (et + 1) * 128],
                        attn_tiles[hp],
                        start=(hp == 0),
                        stop=(hp == NP - 1),
                    )
                oc = opool.tile([128, QC], F32, tag="oc")
                nc.vector.tensor_copy(oc, pc)
                nc.sync.dma_start(out=outT_d[et * 128 : (et + 1) * 128, jsl], in_=oc)

        # ---- ScalarE-forward schedule ----
        # Row order (default [2, 3, 1, 0]): the first row needs only its K
        # prefix + its own Q chunk (~27us of PE), after which exp runs
        # continuously; later rows' remaining prerequisites and the output
        # projections are projected/drained by the PE inside the exp-bound
        # windows.  The last V chunk of the first row is emitted as filler
        # inside the row so the row's scores outrank it in PE priority.
        order = [int(c) for c in ROWORD]
        k_done = -1
        v_done = -1

        def need_k(upto):
            nonlocal k_done
            for c in range(k_done + 1, upto + 1):
                proj_qk("k", KT_sb, bk_sb, c)
            k_done = max(k_done, upto)

        def need_v(upto):
            nonlocal v_done
            for c in range(v_done + 1, upto + 1):
                proj_v(c)
            v_done = max(v_done, upto)

        r0 = order[0]
        need_k(r0)
        proj_qk("q", QT_sb, bq_sb, r0)
        need_v(r0 - 1)
        # bulky non-urgent DMAs issue after the first chunks' loads
        nc.sync.dma_start(
            out=wo_sb, in_=woT_d[:, :].rearrange("(hp p) e -> p hp e", p=128)
        )
        fill = {1: lambda: need_v(r0)} if r0 > 0 else None
        if r0 == 0:
            need_v(0)
        ats = {r0: attn_row(r0, fillers=fill)}
        prev = r0
        for r in order[1:]:
            need_k(r)
            proj_qk("q", QT_sb, bq_sb, r)
            need_v(r)
            outproj_row(prev, ats.pop(prev))
            ats[r] = attn_row(r)
            prev = r
        outproj_row(prev, ats.pop(prev))


_NC_CACHE = {}


def _get_nc(reps: int = 1):
    if reps not in _NC_CACHE:
        _NC_CACHE[reps] = build(reps)
    return _NC_CACHE[reps]


def make_in_maps(q, k, v, wq, bq, wk, bk, wv, bv, wo):
    """Host-side sharding: returns the 8 per-core input dicts."""
    f32 = np.float32
    mdt = NP_MDT
    masks = np.zeros((4, 128, QC), mdt)
    for dd in range(4):
        kl = np.arange(128)[:, None]
        ql = np.arange(QC)[None, :]
        masks[dd] = (128 * dd + kl <= ql).astype(mdt)

    per_batch = []
    for b in range(B):
        per_batch.append(
            (
                np.ascontiguousarray(q[b].T).astype(mdt, copy=False),
                np.ascontiguousarray(k[b].T).astype(mdt, copy=False),
                np.ascontiguousarray(v[b].T).astype(mdt, copy=False),
            )
        )
    per_tp = []
    for t in range(TP):
        C = slice(t * DL, (t + 1) * DL)
        wqT = np.ascontiguousarray(wq[C, :].T).astype(mdt, copy=False)
        wkT = np.ascontiguousarray(wk[C, :].T).astype(mdt, copy=False)
        wvT = np.ascontiguousarray(wv[C, :].T).astype(mdt, copy=False)
        woT = np.ascontiguousarray(wo[:, C].T).astype(mdt, copy=False)
        bq_r = np.ascontiguousarray(bq[C].reshape(NP, 128).T).astype(f32, copy=False)
        bk_r = np.ascontiguousarray(bk[C].reshape(NP, 128).T).astype(f32, copy=False)
        bv_r = np.broadcast_to(
            bv[C].reshape(HL, DK)[None, :, :], (128, HL, DK)
        ).astype(f32)
        per_tp.append((wqT, wkT, wvT, woT, bq_r, bk_r, bv_r))

    in_maps = []
    for c in range(NCORES):
        b, t = c // TP, c % TP
        qT, kT, vT = per_batch[b]
        wqT, wkT, wvT, woT, bq_r, bk_r, bv_r = per_tp[t]
        in_maps.append(
            {
                "qT": qT, "kT": kT, "vT": vT,
                "wqT": wqT, "wkT": wkT, "wvT": wvT, "woT": woT,
                "bq_r": bq_r, "bk_r": bk_r, "bv_r": bv_r,
                "masks": masks, "ones_r": np.ones((128, HL), mdt),
            }
        )
    return in_maps


def kernel(q, k, v, mask, wq, bq, wk, bk, wv, bv, wo, bo):
    q = np.asarray(q, np.float32)
    k = np.asarray(k, np.float32)
    v = np.asarray(v, np.float32)
    wq, bq = np.asarray(wq, np.float32), np.asarray(bq, np.float32)
    wk, bk = np.asarray(wk, np.float32), np.asarray(bk, np.float32)
    wv, bv = np.asarray(wv, np.float32), np.asarray(bv, np.float32)
    wo, bo = np.asarray(wo, np.float32), np.asarray(bo, np.float32)

    nc = _get_nc(1)
    in_maps = make_in_maps(q, k, v, wq, bq, wk, bk, wv, bv, wo)
    res = run_bass_kernel_spmd(nc, in_maps, list(range(NCORES)))

    out = np.empty((B, S, D), np.float32)
    for b in range(B):
        acc = res.results[TP * b]["outT"].astype(np.float32)
        for t in range(1, TP):
            acc = acc + res.results[TP * b + t]["outT"]
        out[b] = acc.T + bo[None, :]
    return out



# revision 19
# speedup vs baseline: 1.0461x; 1.0461x over previous
"""Trainium2 Bass kernel for a causal multi-head-attention block (v5).

Reference computation (B=4, S=2048, D=1024, H=16, DK=64), torch Linear
convention (x @ W.T + b):
    Q = q @ wq.T + bq ; K = k @ wk.T + bk ; V = v @ wv.T + bv
    per head: attn = softmax(mask(Q K^T / sqrt(DK))) ; x = attn @ V
    out = concat_heads(x) @ wo.T + bo
Sharding: 8 cores = data-parallel over batch (4) x tensor-parallel over
heads (2); host sums the two TP partial outputs per batch and adds bo.

Numerics: fp16 matmuls with fp32 PSUM/softmax denominators (rel err
7.1e-04).  fp8/DoubleRow was tried and measured fast (266us) but is
mathematically unviable here: with random (untrained) weights the
attention output shrinks ~1/sqrt(k_eff), the same factor that averages
the per-weight quantization noise, so each fp8 operand's full rms noise
(e4m3 ~2%, e5m2 ~5%) lands in the output (CPU-emulated 6.6e-02 vs the
2e-02 budget).

v5 structural changes over v2 (the 395us fp16 baseline):
  - One 3D-AP DMA per x-chunk / weight tensor instead of 8 per-k-tile
    DMAs: the ~600ns fixed per-descriptor cost dominated; the DMA queue
    had grown to a 40us serialized prefix.
  - Preamble constants (biases/masks/ones) go on the second HWDGE queue
    (Activation's) so the SP queue leads with critical-path x/w bytes.
  - K-chunk 0 and the first row's Q chunk are projected first, so the
    first scores fire after 2 chunk projections instead of 4.
  - The previous row's output projection is drained as per-unit fillers
    INSIDE the next attention row (one et-tile every other k-tile unit):
    emitting it as a block before the row made the PE run 32 matmuls at
    the row boundary while ScalarE starved (sim: ~6us bubble per row).
  - Output partials evacuate as fp16 (host sums TP halves in fp32) -
    halves the output DMA; evacuation copies stay on DVE (gpsimd cannot
    touch PSUM, walrus rejects it; DMA cannot read PSUM either).
  - softmax denominators use the exact nc.vector.reciprocal:
    reciprocal_approx_fast (custom DVE ucode) produced NaN outputs on
    this hardware deployment despite passing CoreSim, whose numpy model
    of the op is not the ucode.
  - attn@V trails scores/exp by PEND=4 k-tiles (HW-measured optimum
    from v2); the diagonal-block mask multiply keeps the stride-0
    head-broadcast AP (HW-measured 576 vs 980 ns).

On-chip dataflow is fully transposed (no on-chip transposes):
  - QT/KT computed as [dq, s] tiles (head pairs on partitions)
  - scoresT[k, q] = KT.T-tile @ QT (two heads row-tiled on the PE array)
  - exp directly out of PSUM on ScalarE (scores are O(6): exp is safely
    bounded in fp32/fp16)
  - attn@V with stationary V_aug [k, 65] whose 65th column of ones
    yields the softmax denominator for free
  - output projection consumes the [d, q] layout directly, emits outT
"""

import sys

sys.path.insert(0, "/opt/trn_rl_repo")

import numpy as np

import concourse.bass as bass
import concourse.mybir as mybir
import concourse.tile as tile
from concourse import bacc
from concourse.bass_utils import run_bass_kernel_spmd

B, S, D, H, DK = 4, 2048, 1024, 16, 64
NCORES = 8
TP = 2  # tensor-parallel ways (head split)
HL = H // TP  # 8 local heads
DL = D // TP  # 512 local projection dims
QC = 512  # q-chunk (moving free dim)
NS = S // QC  # 4 q-chunks
NI = D // 128  # 8 contraction tiles for projections
NP = DL // 128  # 4 head pairs per core
NT = S // 128  # 16 k-tiles
F32 = mybir.dt.float32
F16 = mybir.dt.float16
ACTF = mybir.ActivationFunctionType

import os

MDT = F16
NP_MDT = np.float16
PEND = int(os.environ.get("KERNEL_PEND", "4"))  # attn@V trail depth (k-tiles)
MASKPOOL = int(os.environ.get("KERNEL_MASKPOOL", "0"))  # mask muls on GpSimd
ROWORD = os.environ.get("KERNEL_ROWORD", "2310")  # attention row order
OPACE = int(os.environ.get("KERNEL_OPACE", "2"))  # outproj filler pacing (units)


def build(reps: int = 1):
    """Build + compile the per-core Bass program (same program on all cores)."""
    nc = bacc.Bacc("TRN2", target_bir_lowering=False, num_devices=NCORES)

    qT_d = nc.declare_dram_parameter("qT", [D, S], MDT, isOutput=False)
    kT_d = nc.declare_dram_parameter("kT", [D, S], MDT, isOutput=False)
    vT_d = nc.declare_dram_parameter("vT", [D, S], MDT, isOutput=False)
    wqT_d = nc.declare_dram_parameter("wqT", [D, DL], MDT, isOutput=False)
    wkT_d = nc.declare_dram_parameter("wkT", [D, DL], MDT, isOutput=False)
    wvT_d = nc.declare_dram_parameter("wvT", [D, DL], MDT, isOutput=False)
    woT_d = nc.declare_dram_parameter("woT", [DL, D], MDT, isOutput=False)
    bq_d = nc.declare_dram_parameter("bq_r", [128, NP], F32, isOutput=False)
    bk_d = nc.declare_dram_parameter("bk_r", [128, NP], F32, isOutput=False)
    bv_d = nc.declare_dram_parameter("bv_r", [128, HL, DK], F32, isOutput=False)
    mk_d = nc.declare_dram_parameter("masks", [4, 128, QC], MDT, isOutput=False)
    ones_d = nc.declare_dram_parameter("ones_r", [128, HL], MDT, isOutput=False)
    outT_d = nc.declare_dram_parameter("outT", [D, S], MDT, isOutput=True)

    with tile.TileContext(nc) as tc:
        _emit(nc, tc, reps, qT_d, kT_d, vT_d, wqT_d, wkT_d, wvT_d, woT_d,
              bq_d, bk_d, bv_d, mk_d, ones_d, outT_d)
    nc.compile()
    return nc


def _emit(nc, tc, reps, *args):
    def body():
        _emit_once(nc, tc, *args)

    if reps == 1:
        body()
    else:
        with tc.For_i(0, reps, 1):
            body()


def _emit_once(nc, tc, qT_d, kT_d, vT_d, wqT_d, wkT_d, wvT_d, woT_d,
               bq_d, bk_d, bv_d, mk_d, ones_d, outT_d):
    from collections import deque
    from contextlib import ExitStack

    qT = qT_d[:, :].rearrange("(i p) s -> p i s", p=128)
    kT = kT_d[:, :].rearrange("(i p) s -> p i s", p=128)
    vT = vT_d[:, :].rearrange("(i p) s -> p i s", p=128)
    wT_r = {
        "q": wqT_d[:, :].rearrange("(i p) m -> p i m", p=128),
        "k": wkT_d[:, :].rearrange("(i p) m -> p i m", p=128),
        "v": wvT_d[:, :].rearrange("(i p) m -> p i m", p=128),
    }
    xT_r = {"q": qT, "k": kT, "v": vT}

    with ExitStack() as stack:
        persist = stack.enter_context(tc.tile_pool(name="persist", bufs=1))
        xpool = stack.enter_context(tc.tile_pool(name="xpool", bufs=3))
        epool = stack.enter_context(tc.tile_pool(name="epool", bufs=10))
        apool = stack.enter_context(tc.tile_pool(name="apool", bufs=8))
        rpool = stack.enter_context(tc.tile_pool(name="rpool", bufs=4))
        opool = stack.enter_context(tc.tile_pool(name="opool", bufs=3))
        ps_s = stack.enter_context(tc.tile_pool(name="ps_s", bufs=2, space="PSUM"))
        ps_o = stack.enter_context(tc.tile_pool(name="ps_o", bufs=1, space="PSUM"))
        ps_g = stack.enter_context(tc.tile_pool(name="ps_g", bufs=2, space="PSUM"))

        # Persistent activations: QT/KT [128(=pair dims), NP, S], V_aug.
        QT_sb = persist.tile([128, NP, S], MDT)
        KT_sb = persist.tile([128, NP, S], MDT)
        # V_aug: per k-tile, 8 heads x (64 V cols + 1 ones col)
        V_sb = persist.tile([128, NT, HL * (DK + 1)], MDT)
        v_view = V_sb.rearrange("p t (h j) -> p t h j", j=DK + 1)
        bq_sb = persist.tile([128, NP], F32)
        bk_sb = persist.tile([128, NP], F32)
        bv_sb = persist.tile([128, HL, DK], F32)
        ones_sb = persist.tile([128, HL], MDT)
        mask_sb = persist.tile([128, 4, QC], MDT)
        wo_sb = persist.tile([128, NP, D], MDT)
        wq_sb = persist.tile([128, NI, DL], MDT)
        wk_sb = persist.tile([128, NI, DL], MDT)
        wv_sb = persist.tile([128, NI, DL], MDT)
        w_sb = {"q": wq_sb, "k": wk_sb, "v": wv_sb}

        # preamble constants go on the second HWDGE queue (Activation's) so
        # the SP queue leads with the critical-path x/w bytes
        nc.scalar.dma_start(out=bq_sb, in_=bq_d[:, :])
        nc.scalar.dma_start(out=bk_sb, in_=bk_d[:, :])
        nc.scalar.dma_start(out=bv_sb, in_=bv_d[:, :, :])
        nc.scalar.dma_start(out=ones_sb, in_=ones_d[:, :])
        nc.scalar.dma_start(out=mask_sb, in_=mk_d[:, :, :].rearrange("d p q -> p d q"))
        # ones columns of V_aug (written once; disjoint from the V writes)
        for t in range(NT):
            nc.gpsimd.tensor_copy(v_view[:, t, :, DK], ones_sb)

        w_loaded = set()

        def load_chunk(which, sc):
            """x-chunk DMA (one 3D-AP transfer: the ~600ns fixed per-DMA cost
            dominates); the weight DMA issues on first use of each
            projection."""
            ssl = slice(sc * QC, (sc + 1) * QC)
            x = xpool.tile([128, NI, QC], MDT, tag="xq")
            nc.sync.dma_start(out=x, in_=xT_r[which][:, :, ssl])
            if which not in w_loaded:
                w_loaded.add(which)
                nc.sync.dma_start(out=w_sb[which], in_=wT_r[which][:, :, :])
            return x

        def proj_qk(which, dst, bias, sc):
            ssl = slice(sc * QC, (sc + 1) * QC)
            x = load_chunk(which, sc)
            for hp in range(NP):
                psum = ps_g.tile([128, QC], F32, tag="pg")
                for it in range(NI):
                    nc.tensor.matmul(
                        psum,
                        w_sb[which][:, it, hp * 128 : (hp + 1) * 128],
                        x[:, it, :],
                        start=(it == 0),
                        stop=(it == NI - 1),
                    )
                nc.vector.tensor_scalar_add(
                    dst[:, hp, ssl], psum, bias[:, hp : hp + 1]
                )

        def proj_v(sc):
            x = load_chunk("v", sc)
            for st4 in range(4):
                st = 4 * sc + st4
                psum = ps_g.tile([128, QC], F32, tag="pg")
                for it in range(NI):
                    nc.tensor.matmul(
                        psum,
                        x[:, it, st4 * 128 : (st4 + 1) * 128],
                        wv_sb[:, it, :],
                        start=(it == 0),
                        stop=(it == NI - 1),
                    )
                # scatter head columns into the 65-stride V_aug layout,
                # adding the bias in the same pass
                nc.vector.tensor_add(
                    v_view[:, st, :, 0:DK],
                    psum.rearrange("p (h d) -> p h d", d=DK),
                    bv_sb,
                )

        def attn_row(j, fillq=None):
            """Attention row j (q-chunk j over k-tiles 0..4(j+1)-1);
            returns the normalized attn tiles for the output projection.
            fillq: deque of PE-filler callbacks (projections / the previous
            row's output projection); one is drained every OPACE k-tile
            units, popped BEFORE the trailing attn@V so a filler that
            produces data an attn@V needs is always emitted ahead of it."""
            nkt = 4 * (j + 1)
            attn_tiles = []
            drain_tick = 0

            def drain_fill():
                nonlocal drain_tick
                drain_tick += 1
                if fillq and drain_tick % OPACE == 0:
                    fillq.popleft()()

            for hp in range(NP):
                po2 = ps_o.tile([DK + 1, 2, QC], F32, tag="po")
                # software pipeline: attn@V trails scores/exp by PEND k-tiles
                # so the PE never waits on the just-issued exp
                pend = []

                def attnv(kt, e0, e1, off):
                    nc.tensor.matmul(
                        po2[:, 0, off:],
                        v_view[:, kt, 2 * hp, :],
                        e0,
                        start=(kt == 0),
                        stop=(kt == nkt - 1),
                    )
                    nc.tensor.matmul(
                        po2[:, 1, off:],
                        v_view[:, kt, 2 * hp + 1, :],
                        e1,
                        start=(kt == 0),
                        stop=(kt == nkt - 1),
                    )

                for kt in range(nkt):
                    ksl = slice(kt * 128, (kt + 1) * 128)
                    dd = kt - 4 * j
                    # diagonal blocks only have live queries at q >= 128*dd
                    off = 128 * dd if dd > 0 else 0
                    jssl = slice(j * QC + off, (j + 1) * QC)
                    # both heads' score blocks in one 2-bank PSUM tile so
                    # a single [128, 2, N'] exp covers the pair
                    ps2 = ps_s.tile([128, 2, QC], F32, tag="ps")
                    nc.tensor.matmul(
                        ps2[:, 0, off:],
                        KT_sb[0:64, hp, ksl],
                        QT_sb[0:64, hp, jssl],
                        start=True,
                        stop=True,
                    )
                    nc.tensor.matmul(
                        ps2[:, 1, off:],
                        KT_sb[64:128, hp, ksl],
                        QT_sb[64:128, hp, jssl],
                        start=True,
                        stop=True,
                        tile_position=(64, 0),
                    )
                    ex2 = epool.tile([128, 2, QC], MDT, tag="ex")
                    nc.scalar.activation(
                        ex2[:, :, off:], ps2[:, :, off:], ACTF.Exp, scale=1.0 / 8.0
                    )
                    if dd >= 0:  # diagonal block: zero the k > q entries
                        # stride-0 head-broadcast AP: measured FASTER on HW
                        # than a dense fp16 operand (576 vs 980 ns)
                        m = mask_sb[:, dd, off:]
                        mb = bass.AP(
                            tensor=m.tensor, offset=m.offset,
                            ap=[list(m.ap[0]), [0, 2], list(m.ap[1])],
                        )
                        eng = nc.gpsimd if MASKPOOL else nc.vector
                        eng.tensor_mul(ex2[:, :, off:], ex2[:, :, off:], mb)
                    pend.append((kt, ex2[:, 0, off:], ex2[:, 1, off:], off))
                    drain_fill()
                    if len(pend) > PEND:
                        attnv(*pend.pop(0))
                for p_ in pend:
                    attnv(*p_)
                # evacuate both accumulators in one op so the ps_o banks
                # free for the next pair; normalize from SBUF
                ub2 = rpool.tile([DK + 1, 2, QC], F32, tag="ub")
                nc.vector.tensor_copy(ub2, po2)
                # normalize: row DK of ub2 is the softmax denominator.
                # approx reciprocal: ~18 correct bits, 5x faster; denominators
                # are >= exp(-8) so the denorm/inf edge cases can't occur
                rec = rpool.tile([1, 2, QC], F32, tag="rec")
                nc.vector.reciprocal(rec, ub2[DK : DK + 1, :, :])
                rb = rpool.tile([64, 2, QC], F32, tag="rb")
                nc.gpsimd.partition_broadcast(rb, rec)
                attn = apool.tile([128, QC], MDT, tag="attn")
                nc.vector.tensor_mul(attn[0:64, :], ub2[0:64, 0, :], rb[:, 0, :])
                # h1 written straight to partitions 64..127 (DVE operands
                # may sit at different base partitions)
                nc.vector.tensor_mul(attn[64:128, :], ub2[0:64, 1, :], rb[:, 1, :])
                attn_tiles.append(attn)
            return attn_tiles

        def outproj_et(j, attn_tiles, et):
            """One output-feature tile of row j's output projection."""
            jsl = slice(j * QC, (j + 1) * QC)
            pc = ps_g.tile([128, QC], F32, tag="pg")
            for hp in range(NP):
                nc.tensor.matmul(
                    pc,
                    wo_sb[:, hp, et * 128 : (et + 1) * 128],
                    attn_tiles[hp],
                    start=(hp == 0),
                    stop=(hp == NP - 1),
                )
            # fp16 evacuation halves the output DMA; host sums TP partials
            # in fp32 (DVE: neither gpsimd nor DMA can read PSUM)
            oc = opool.tile([128, QC], MDT, tag="oc")
            nc.vector.tensor_copy(oc, pc)
            nc.sync.dma_start(out=outT_d[et * 128 : (et + 1) * 128, jsl], in_=oc)

        def outproj_row(j, attn_tiles):
            for et in range(NI):
                outproj_et(j, attn_tiles, et)

        # ---- ScalarE-forward schedule ----
        # Row order (default [2, 3, 1, 0]): the first row needs only K-chunk
        # 0 + its own Q chunk for its first scores (projected first; the
        # remaining K chunks follow), after which exp runs continuously;
        # later rows' prerequisites are projected inside the previous row's
        # exp-bound windows, and the previous row's output projection drains
        # as per-unit fillers inside the NEXT row so it never outranks that
        # row's scores at the row boundary.
        order = [int(c) for c in ROWORD]
        k_done = -1
        v_done = -1

        def need_k(upto):
            nonlocal k_done
            for c in range(k_done + 1, upto + 1):
                proj_qk("k", KT_sb, bk_sb, c)
            k_done = max(k_done, upto)

        def need_v(upto):
            nonlocal v_done
            for c in range(v_done + 1, upto + 1):
                proj_v(c)
            v_done = max(v_done, upto)

        r0 = order[0]
        need_k(0)
        proj_qk("q", QT_sb, bq_sb, r0)
        need_k(r0)
        need_v(r0 - 1)
        # bulky non-urgent DMAs issue after the first chunks' loads
        nc.sync.dma_start(
            out=wo_sb, in_=woT_d[:, :].rearrange("(hp p) e -> p hp e", p=128)
        )
        fillq = deque([lambda: need_v(r0)] if r0 > 0 else [])
        if r0 == 0:
            need_v(0)
        ats = {r0: attn_row(r0, fillq)}
        prev = r0
        for r in order[1:]:
            need_k(r)
            proj_qk("q", QT_sb, bq_sb, r)
            need_v(r)
            pats = ats.pop(prev)
            fillq = deque(
                (lambda j=prev, a=pats, et=et: outproj_et(j, a, et))
                for et in range(NI)
            )
            ats[r] = attn_row(r, fillq)
            # any pieces the row didn't drain (short rows) are emitted now
            while fillq:
                fillq.popleft()()
            prev = r
        outproj_row(prev, ats.pop(prev))


_NC_CACHE = {}


def _get_nc(reps: int = 1):
    if reps not in _NC_CACHE:
        _NC_CACHE[reps] = build(reps)
    return _NC_CACHE[reps]


def make_in_maps(q, k, v, wq, bq, wk, bk, wv, bv, wo):
    """Host-side sharding: returns the 8 per-core input dicts."""
    f32 = np.float32
    mdt = NP_MDT
    masks = np.zeros((4, 128, QC), mdt)
    for dd in range(4):
        kl = np.arange(128)[:, None]
        ql = np.arange(QC)[None, :]
        masks[dd] = (128 * dd + kl <= ql).astype(mdt)

    per_batch = []
    for b in range(B):
        per_batch.append(
            (
                np.ascontiguousarray(q[b].T).astype(mdt, copy=False),
                np.ascontiguousarray(k[b].T).astype(mdt, copy=False),
                np.ascontiguousarray(v[b].T).astype(mdt, copy=False),
            )
        )
    per_tp = []
    for t in range(TP):
        C = slice(t * DL, (t + 1) * DL)
        wqT = np.ascontiguousarray(wq[C, :].T).astype(mdt, copy=False)
        wkT = np.ascontiguousarray(wk[C, :].T).astype(mdt, copy=False)
        wvT = np.ascontiguousarray(wv[C, :].T).astype(mdt, copy=False)
        woT = np.ascontiguousarray(wo[:, C].T).astype(mdt, copy=False)
        bq_r = np.ascontiguousarray(bq[C].reshape(NP, 128).T).astype(f32, copy=False)
        bk_r = np.ascontiguousarray(bk[C].reshape(NP, 128).T).astype(f32, copy=False)
        bv_r = np.broadcast_to(
            bv[C].reshape(HL, DK)[None, :, :], (128, HL, DK)
        ).astype(f32)
        per_tp.append((wqT, wkT, wvT, woT, bq_r, bk_r, bv_r))

    in_maps = []
    for c in range(NCORES):
        b, t = c // TP, c % TP
        qT, kT, vT = per_batch[b]
        wqT, wkT, wvT, woT, bq_r, bk_r, bv_r = per_tp[t]
        in_maps.append(
            {
                "qT": qT, "kT": kT, "vT": vT,
                "wqT": wqT, "wkT": wkT, "wvT": wvT, "woT": woT,
                "bq_r": bq_r, "bk_r": bk_r, "bv_r": bv_r,
                "masks": masks, "ones_r": np.ones((128, HL), mdt),
            }
        )
    return in_maps


def kernel(q, k, v, mask, wq, bq, wk, bk, wv, bv, wo, bo):
    q = np.asarray(q, np.float32)
    k = np.asarray(k, np.float32)
    v = np.asarray(v, np.float32)
    wq, bq = np.asarray(wq, np.float32), np.asarray(bq, np.float32)
    wk, bk = np.asarray(wk, np.float32), np.asarray(bk, np.float32)
    wv, bv = np.asarray(wv, np.float32), np.asarray(bv, np.float32)
    wo, bo = np.asarray(wo, np.float32), np.asarray(bo, np.float32)

    nc = _get_nc(1)
    in_maps = make_in_maps(q, k, v, wq, bq, wk, bk, wv, bv, wo)
    res = run_bass_kernel_spmd(nc, in_maps, list(range(NCORES)))

    out = np.empty((B, S, D), np.float32)
    for b in range(B):
        acc = res.results[TP * b]["outT"].astype(np.float32)
        for t in range(1, TP):
            acc = acc + res.results[TP * b + t]["outT"].astype(np.float32)
        out[b] = acc.T + bo[None, :]
    return out


# revision 22
# speedup vs baseline: 1.1963x; 1.1436x over previous
"""Trainium2 Bass kernel for a causal multi-head-attention block (v5).

Reference computation (B=4, S=2048, D=1024, H=16, DK=64), torch Linear
convention (x @ W.T + b):
    Q = q @ wq.T + bq ; K = k @ wk.T + bk ; V = v @ wv.T + bv
    per head: attn = softmax(mask(Q K^T / sqrt(DK))) ; x = attn @ V
    out = concat_heads(x) @ wo.T + bo
Sharding: 8 cores = data-parallel over batch (4) x tensor-parallel over
heads (2); host sums the two TP partial outputs per batch and adds bo.

Numerics: fp16 matmuls with fp32 PSUM/softmax denominators (rel err
7.1e-04).  fp8/DoubleRow was tried and measured fast (266us) but is
mathematically unviable here: with random (untrained) weights the
attention output shrinks ~1/sqrt(k_eff), the same factor that averages
the per-weight quantization noise, so each fp8 operand's full rms noise
(e4m3 ~2%, e5m2 ~5%) lands in the output (CPU-emulated 6.6e-02 vs the
2e-02 budget).

v5 structural changes over v2 (the 395us fp16 baseline):
  - One 3D-AP DMA per x-chunk / weight tensor instead of 8 per-k-tile
    DMAs: the ~600ns fixed per-descriptor cost dominated; the DMA queue
    had grown to a 40us serialized prefix.
  - Preamble constants (biases/masks/ones) go on the second HWDGE queue
    (Activation's) so the SP queue leads with critical-path x/w bytes.
  - K-chunk 0 and the first row's Q chunk are projected first, so the
    first scores fire after 2 chunk projections instead of 4.
  - The previous row's output projection is drained as per-unit fillers
    INSIDE the next attention row (one et-tile every other k-tile unit):
    emitting it as a block before the row made the PE run 32 matmuls at
    the row boundary while ScalarE starved (sim: ~6us bubble per row).
  - Output partials evacuate as fp16 (host sums TP halves in fp32) -
    halves the output DMA; evacuation copies stay on DVE (gpsimd cannot
    touch PSUM, walrus rejects it; DMA cannot read PSUM either).
  - softmax denominators use the exact nc.vector.reciprocal:
    reciprocal_approx_fast (custom DVE ucode) produced NaN outputs on
    this hardware deployment despite passing CoreSim, whose numpy model
    of the op is not the ucode.
  - attn@V trails scores/exp by PEND=4 k-tiles (HW-measured optimum
    from v2); the diagonal-block mask multiply keeps the stride-0
    head-broadcast AP (HW-measured 576 vs 980 ns).

On-chip dataflow is fully transposed (no on-chip transposes):
  - QT/KT computed as [dq, s] tiles (head pairs on partitions)
  - scoresT[k, q] = KT.T-tile @ QT (two heads row-tiled on the PE array)
  - exp directly out of PSUM on ScalarE (scores are O(6): exp is safely
    bounded in fp32/fp16)
  - attn@V with stationary V_aug [k, 65] whose 65th column of ones
    yields the softmax denominator for free
  - output projection consumes the [d, q] layout directly, emits outT
"""

import sys

sys.path.insert(0, "/opt/trn_rl_repo")

import numpy as np

import concourse.bass as bass
import concourse.mybir as mybir
import concourse.tile as tile
from concourse import bacc
from concourse.bass_utils import run_bass_kernel_spmd

B, S, D, H, DK = 4, 2048, 1024, 16, 64
NCORES = 8
TP = 2  # tensor-parallel ways (head split)
HL = H // TP  # 8 local heads
DL = D // TP  # 512 local projection dims
QC = 512  # q-chunk (moving free dim)
NS = S // QC  # 4 q-chunks
NI = D // 128  # 8 contraction tiles for projections
NP = DL // 128  # 4 head pairs per core
NT = S // 128  # 16 k-tiles
F32 = mybir.dt.float32
F16 = mybir.dt.float16
ACTF = mybir.ActivationFunctionType

import os

MDT = F16
NP_MDT = np.float16
PEND = int(os.environ.get("KERNEL_PEND", "4"))  # attn@V trail depth (k-tiles)
MASKPOOL = int(os.environ.get("KERNEL_MASKPOOL", "0"))  # mask muls on GpSimd
ROWORD = os.environ.get("KERNEL_ROWORD", "2310")  # attention row order
OPACE = int(os.environ.get("KERNEL_OPACE", "2"))  # outproj filler pacing (units)


def build(reps: int = 1):
    """Build + compile the per-core Bass program (same program on all cores)."""
    nc = bacc.Bacc("TRN2", target_bir_lowering=False, num_devices=NCORES)

    qT_d = nc.declare_dram_parameter("qT", [D, S], MDT, isOutput=False)
    kT_d = nc.declare_dram_parameter("kT", [D, S], MDT, isOutput=False)
    vT_d = nc.declare_dram_parameter("vT", [D, S], MDT, isOutput=False)
    wqT_d = nc.declare_dram_parameter("wqT", [D, DL], MDT, isOutput=False)
    wkT_d = nc.declare_dram_parameter("wkT", [D, DL], MDT, isOutput=False)
    wvT_d = nc.declare_dram_parameter("wvT", [D, DL], MDT, isOutput=False)
    woT_d = nc.declare_dram_parameter("woT", [DL, D], MDT, isOutput=False)
    bq_d = nc.declare_dram_parameter("bq_r", [128, NP], F32, isOutput=False)
    bk_d = nc.declare_dram_parameter("bk_r", [128, NP], F32, isOutput=False)
    bv_d = nc.declare_dram_parameter("bv_r", [128, HL, DK], F32, isOutput=False)
    mk_d = nc.declare_dram_parameter("masks", [4, 128, QC], MDT, isOutput=False)
    ones_d = nc.declare_dram_parameter("ones_r", [128, HL], MDT, isOutput=False)
    outT_d = nc.declare_dram_parameter("outT", [D, S], MDT, isOutput=True)

    with tile.TileContext(nc) as tc:
        _emit(nc, tc, reps, qT_d, kT_d, vT_d, wqT_d, wkT_d, wvT_d, woT_d,
              bq_d, bk_d, bv_d, mk_d, ones_d, outT_d)

    # The kernel uses exactly {Exp, Ln}; both live in the
    # natural_log_exp_and_others table set, but the table-load pass
    # assigns each function its first-listed home set, alternating between
    # exp_and_others and natural_log (32 x 2.7us table switches).  During
    # compile only, present a view of the tables where the combined set is
    # the sole home of Exp/Ln (dict order, and hence act_func_set_ids, is
    # unchanged) so one table load covers the whole kernel.
    _orig_gat = bacc.get_activation_tables
    _target = "natural_log_exp_and_others"

    def _gat(arch):
        tabs = _orig_gat(arch)
        if _target in tabs and {
            mybir.ActivationFunctionType.Exp,
            mybir.ActivationFunctionType.Ln,
        } <= tabs[_target]:
            for name, funcs in tabs.items():
                if name != _target:
                    funcs.discard(mybir.ActivationFunctionType.Exp)
                    funcs.discard(mybir.ActivationFunctionType.Ln)
        return tabs

    bacc.get_activation_tables = _gat
    try:
        nc.compile()
    finally:
        bacc.get_activation_tables = _orig_gat
    return nc


def _emit(nc, tc, reps, *args):
    def body():
        _emit_once(nc, tc, *args)

    if reps == 1:
        body()
    else:
        with tc.For_i(0, reps, 1):
            body()


def _emit_once(nc, tc, qT_d, kT_d, vT_d, wqT_d, wkT_d, wvT_d, woT_d,
               bq_d, bk_d, bv_d, mk_d, ones_d, outT_d):
    from collections import deque
    from contextlib import ExitStack

    qT = qT_d[:, :].rearrange("(i p) s -> p i s", p=128)
    kT = kT_d[:, :].rearrange("(i p) s -> p i s", p=128)
    vT = vT_d[:, :].rearrange("(i p) s -> p i s", p=128)
    wT_r = {
        "q": wqT_d[:, :].rearrange("(i p) m -> p i m", p=128),
        "k": wkT_d[:, :].rearrange("(i p) m -> p i m", p=128),
        "v": wvT_d[:, :].rearrange("(i p) m -> p i m", p=128),
    }
    xT_r = {"q": qT, "k": kT, "v": vT}

    with ExitStack() as stack:
        persist = stack.enter_context(tc.tile_pool(name="persist", bufs=1))
        xpool = stack.enter_context(tc.tile_pool(name="xpool", bufs=3))
        epool = stack.enter_context(tc.tile_pool(name="epool", bufs=10))
        apool = stack.enter_context(tc.tile_pool(name="apool", bufs=8))
        rpool = stack.enter_context(tc.tile_pool(name="rpool", bufs=4))
        opool = stack.enter_context(tc.tile_pool(name="opool", bufs=3))
        ps_s = stack.enter_context(tc.tile_pool(name="ps_s", bufs=2, space="PSUM"))
        ps_o = stack.enter_context(tc.tile_pool(name="ps_o", bufs=1, space="PSUM"))
        ps_g = stack.enter_context(tc.tile_pool(name="ps_g", bufs=2, space="PSUM"))

        # Persistent activations: QT/KT [128(=pair dims), NP, S], V_aug.
        QT_sb = persist.tile([128, NP, S], MDT)
        KT_sb = persist.tile([128, NP, S], MDT)
        # V_aug: per k-tile, 8 heads x (64 V cols + 1 ones col)
        V_sb = persist.tile([128, NT, HL * (DK + 1)], MDT)
        v_view = V_sb.rearrange("p t (h j) -> p t h j", j=DK + 1)
        bq_sb = persist.tile([128, NP], F32)
        bk_sb = persist.tile([128, NP], F32)
        bv_sb = persist.tile([128, HL, DK], F32)
        ones_sb = persist.tile([128, HL], MDT)
        mask_sb = persist.tile([128, 4, QC], MDT)
        wo_sb = persist.tile([128, NP, D], MDT)
        wq_sb = persist.tile([128, NI, DL], MDT)
        wk_sb = persist.tile([128, NI, DL], MDT)
        wv_sb = persist.tile([128, NI, DL], MDT)
        w_sb = {"q": wq_sb, "k": wk_sb, "v": wv_sb}

        # preamble constants go on the second HWDGE queue (Activation's) so
        # the SP queue leads with the critical-path x/w bytes
        nc.scalar.dma_start(out=bq_sb, in_=bq_d[:, :])
        nc.scalar.dma_start(out=bk_sb, in_=bk_d[:, :])
        nc.scalar.dma_start(out=bv_sb, in_=bv_d[:, :, :])
        nc.scalar.dma_start(out=ones_sb, in_=ones_d[:, :])
        nc.scalar.dma_start(out=mask_sb, in_=mk_d[:, :, :].rearrange("d p q -> p d q"))
        # ones columns of V_aug (written once; disjoint from the V writes)
        for t in range(NT):
            nc.gpsimd.tensor_copy(v_view[:, t, :, DK], ones_sb)

        w_loaded = set()

        def load_chunk(which, sc):
            """x-chunk DMA (one 3D-AP transfer: the ~600ns fixed per-DMA cost
            dominates); the weight DMA issues on first use of each
            projection."""
            ssl = slice(sc * QC, (sc + 1) * QC)
            x = xpool.tile([128, NI, QC], MDT, tag="xq")
            nc.sync.dma_start(out=x, in_=xT_r[which][:, :, ssl])
            if which not in w_loaded:
                w_loaded.add(which)
                nc.sync.dma_start(out=w_sb[which], in_=wT_r[which][:, :, :])
            return x

        def proj_qk(which, dst, bias, sc):
            ssl = slice(sc * QC, (sc + 1) * QC)
            x = load_chunk(which, sc)
            for hp in range(NP):
                psum = ps_g.tile([128, QC], F32, tag="pg")
                for it in range(NI):
                    nc.tensor.matmul(
                        psum,
                        w_sb[which][:, it, hp * 128 : (hp + 1) * 128],
                        x[:, it, :],
                        start=(it == 0),
                        stop=(it == NI - 1),
                    )
                nc.vector.tensor_scalar_add(
                    dst[:, hp, ssl], psum, bias[:, hp : hp + 1]
                )

        def proj_v(sc):
            x = load_chunk("v", sc)
            for st4 in range(4):
                st = 4 * sc + st4
                psum = ps_g.tile([128, QC], F32, tag="pg")
                for it in range(NI):
                    nc.tensor.matmul(
                        psum,
                        x[:, it, st4 * 128 : (st4 + 1) * 128],
                        wv_sb[:, it, :],
                        start=(it == 0),
                        stop=(it == NI - 1),
                    )
                # scatter head columns into the 65-stride V_aug layout,
                # adding the bias in the same pass
                nc.vector.tensor_add(
                    v_view[:, st, :, 0:DK],
                    psum.rearrange("p (h d) -> p h d", d=DK),
                    bv_sb,
                )

        def attn_row(j, fillq=None):
            """Attention row j (q-chunk j over k-tiles 0..4(j+1)-1);
            returns the normalized attn tiles for the output projection.
            fillq: deque of PE-filler callbacks (projections / the previous
            row's output projection); one is drained every OPACE k-tile
            units, popped BEFORE the trailing attn@V so a filler that
            produces data an attn@V needs is always emitted ahead of it."""
            nkt = 4 * (j + 1)
            attn_tiles = []
            drain_tick = 0

            def drain_fill():
                nonlocal drain_tick
                drain_tick += 1
                if fillq and drain_tick % OPACE == 0:
                    fillq.popleft()()

            for hp in range(NP):
                po2 = ps_o.tile([DK + 1, 2, QC], F32, tag="po")
                # software pipeline: attn@V trails scores/exp by PEND k-tiles
                # so the PE never waits on the just-issued exp
                pend = []

                def attnv(kt, e0, e1, off):
                    nc.tensor.matmul(
                        po2[:, 0, off:],
                        v_view[:, kt, 2 * hp, :],
                        e0,
                        start=(kt == 0),
                        stop=(kt == nkt - 1),
                    )
                    nc.tensor.matmul(
                        po2[:, 1, off:],
                        v_view[:, kt, 2 * hp + 1, :],
                        e1,
                        start=(kt == 0),
                        stop=(kt == nkt - 1),
                    )

                for kt in range(nkt):
                    ksl = slice(kt * 128, (kt + 1) * 128)
                    dd = kt - 4 * j
                    # diagonal blocks only have live queries at q >= 128*dd
                    off = 128 * dd if dd > 0 else 0
                    jssl = slice(j * QC + off, (j + 1) * QC)
                    # both heads' score blocks in one 2-bank PSUM tile so
                    # a single [128, 2, N'] exp covers the pair
                    ps2 = ps_s.tile([128, 2, QC], F32, tag="ps")
                    nc.tensor.matmul(
                        ps2[:, 0, off:],
                        KT_sb[0:64, hp, ksl],
                        QT_sb[0:64, hp, jssl],
                        start=True,
                        stop=True,
                    )
                    nc.tensor.matmul(
                        ps2[:, 1, off:],
                        KT_sb[64:128, hp, ksl],
                        QT_sb[64:128, hp, jssl],
                        start=True,
                        stop=True,
                        tile_position=(64, 0),
                    )
                    ex2 = epool.tile([128, 2, QC], MDT, tag="ex")
                    nc.scalar.activation(
                        ex2[:, :, off:], ps2[:, :, off:], ACTF.Exp, scale=1.0 / 8.0
                    )
                    if dd >= 0:  # diagonal block: zero the k > q entries
                        # stride-0 head-broadcast AP: measured FASTER on HW
                        # than a dense fp16 operand (576 vs 980 ns)
                        m = mask_sb[:, dd, off:]
                        mb = bass.AP(
                            tensor=m.tensor, offset=m.offset,
                            ap=[list(m.ap[0]), [0, 2], list(m.ap[1])],
                        )
                        eng = nc.gpsimd if MASKPOOL else nc.vector
                        eng.tensor_mul(ex2[:, :, off:], ex2[:, :, off:], mb)
                    pend.append((kt, ex2[:, 0, off:], ex2[:, 1, off:], off))
                    drain_fill()
                    if len(pend) > PEND:
                        attnv(*pend.pop(0))
                for p_ in pend:
                    attnv(*p_)
                # evacuate both accumulators in one op so the ps_o banks
                # free for the next pair; normalize from SBUF
                ub2 = rpool.tile([DK + 1, 2, QC], F32, tag="ub")
                nc.vector.tensor_copy(ub2, po2)
                # normalize: row DK of ub2 is the softmax denominator.
                # 1/den as exp(-ln(den)) on ScalarE: the natural_log_exp
                # table set holds BOTH functions (one table load total), the
                # -1 folds into the activation's free scale, and it takes
                # the reciprocal off DVE, whose exact InstReciprocal costs
                # ~5us per call on HW (reciprocal_approx_fast is ~5x faster
                # but its custom ucode returned NaNs on this deployment)
                lg = rpool.tile([1, 2, QC], F32, tag="lg")
                rec = rpool.tile([1, 2, QC], F32, tag="rec")
                nc.scalar.activation(lg, ub2[DK : DK + 1, :, :], ACTF.Ln)
                nc.scalar.activation(rec, lg, ACTF.Exp, scale=-1.0)
                rb = rpool.tile([64, 2, QC], F32, tag="rb")
                nc.gpsimd.partition_broadcast(rb, rec)
                attn = apool.tile([128, QC], MDT, tag="attn")
                nc.vector.tensor_mul(attn[0:64, :], ub2[0:64, 0, :], rb[:, 0, :])
                # h1 written straight to partitions 64..127 (DVE operands
                # may sit at different base partitions)
                nc.vector.tensor_mul(attn[64:128, :], ub2[0:64, 1, :], rb[:, 1, :])
                attn_tiles.append(attn)
            return attn_tiles

        def outproj_et(j, attn_tiles, et):
            """One output-feature tile of row j's output projection."""
            jsl = slice(j * QC, (j + 1) * QC)
            pc = ps_g.tile([128, QC], F32, tag="pg")
            for hp in range(NP):
                nc.tensor.matmul(
                    pc,
                    wo_sb[:, hp, et * 128 : (et + 1) * 128],
                    attn_tiles[hp],
                    start=(hp == 0),
                    stop=(hp == NP - 1),
                )
            # fp16 evacuation halves the output DMA; host sums TP partials
            # in fp32 (DVE: neither gpsimd nor DMA can read PSUM)
            oc = opool.tile([128, QC], MDT, tag="oc")
            nc.vector.tensor_copy(oc, pc)
            nc.sync.dma_start(out=outT_d[et * 128 : (et + 1) * 128, jsl], in_=oc)

        def outproj_row(j, attn_tiles):
            for et in range(NI):
                outproj_et(j, attn_tiles, et)

        # ---- ScalarE-forward schedule ----
        # Row order (default [2, 3, 1, 0]): the first row needs only K-chunk
        # 0 + its own Q chunk for its first scores (projected first; the
        # remaining K chunks follow), after which exp runs continuously;
        # later rows' prerequisites are projected inside the previous row's
        # exp-bound windows, and the previous row's output projection drains
        # as per-unit fillers inside the NEXT row so it never outranks that
        # row's scores at the row boundary.
        order = [int(c) for c in ROWORD]
        k_done = -1
        v_done = -1

        def need_k(upto):
            nonlocal k_done
            for c in range(k_done + 1, upto + 1):
                proj_qk("k", KT_sb, bk_sb, c)
            k_done = max(k_done, upto)

        def need_v(upto):
            nonlocal v_done
            for c in range(v_done + 1, upto + 1):
                proj_v(c)
            v_done = max(v_done, upto)

        r0 = order[0]
        need_k(0)
        proj_qk("q", QT_sb, bq_sb, r0)
        need_k(r0)
        need_v(r0 - 1)
        # bulky non-urgent DMAs issue after the first chunks' loads
        nc.sync.dma_start(
            out=wo_sb, in_=woT_d[:, :].rearrange("(hp p) e -> p hp e", p=128)
        )
        fillq = deque([lambda: need_v(r0)] if r0 > 0 else [])
        if r0 == 0:
            need_v(0)
        ats = {r0: attn_row(r0, fillq)}
        prev = r0
        for r in order[1:]:
            need_k(r)
            proj_qk("q", QT_sb, bq_sb, r)
            need_v(r)
            pats = ats.pop(prev)
            fillq = deque(
                (lambda j=prev, a=pats, et=et: outproj_et(j, a, et))
                for et in range(NI)
            )
            ats[r] = attn_row(r, fillq)
            # any pieces the row didn't drain (short rows) are emitted now
            while fillq:
                fillq.popleft()()
            prev = r
        outproj_row(prev, ats.pop(prev))


_NC_CACHE = {}


def _get_nc(reps: int = 1):
    if reps not in _NC_CACHE:
        _NC_CACHE[reps] = build(reps)
    return _NC_CACHE[reps]


def make_in_maps(q, k, v, wq, bq, wk, bk, wv, bv, wo):
    """Host-side sharding: returns the 8 per-core input dicts."""
    f32 = np.float32
    mdt = NP_MDT
    masks = np.zeros((4, 128, QC), mdt)
    for dd in range(4):
        kl = np.arange(128)[:, None]
        ql = np.arange(QC)[None, :]
        masks[dd] = (128 * dd + kl <= ql).astype(mdt)

    per_batch = []
    for b in range(B):
        per_batch.append(
            (
                np.ascontiguousarray(q[b].T).astype(mdt, copy=False),
                np.ascontiguousarray(k[b].T).astype(mdt, copy=False),
                np.ascontiguousarray(v[b].T).astype(mdt, copy=False),
            )
        )
    per_tp = []
    for t in range(TP):
        C = slice(t * DL, (t + 1) * DL)
        wqT = np.ascontiguousarray(wq[C, :].T).astype(mdt, copy=False)
        wkT = np.ascontiguousarray(wk[C, :].T).astype(mdt, copy=False)
        wvT = np.ascontiguousarray(wv[C, :].T).astype(mdt, copy=False)
        woT = np.ascontiguousarray(wo[:, C].T).astype(mdt, copy=False)
        bq_r = np.ascontiguousarray(bq[C].reshape(NP, 128).T).astype(f32, copy=False)
        bk_r = np.ascontiguousarray(bk[C].reshape(NP, 128).T).astype(f32, copy=False)
        bv_r = np.broadcast_to(
            bv[C].reshape(HL, DK)[None, :, :], (128, HL, DK)
        ).astype(f32)
        per_tp.append((wqT, wkT, wvT, woT, bq_r, bk_r, bv_r))

    in_maps = []
    for c in range(NCORES):
        b, t = c // TP, c % TP
        qT, kT, vT = per_batch[b]
        wqT, wkT, wvT, woT, bq_r, bk_r, bv_r = per_tp[t]
        in_maps.append(
            {
                "qT": qT, "kT": kT, "vT": vT,
                "wqT": wqT, "wkT": wkT, "wvT": wvT, "woT": woT,
                "bq_r": bq_r, "bk_r": bk_r, "bv_r": bv_r,
                "masks": masks, "ones_r": np.ones((128, HL), mdt),
            }
        )
    return in_maps


def kernel(q, k, v, mask, wq, bq, wk, bk, wv, bv, wo, bo):
    q = np.asarray(q, np.float32)
    k = np.asarray(k, np.float32)
    v = np.asarray(v, np.float32)
    wq, bq = np.asarray(wq, np.float32), np.asarray(bq, np.float32)
    wk, bk = np.asarray(wk, np.float32), np.asarray(bk, np.float32)
    wv, bv = np.asarray(wv, np.float32), np.asarray(bv, np.float32)
    wo, bo = np.asarray(wo, np.float32), np.asarray(bo, np.float32)

    nc = _get_nc(1)
    in_maps = make_in_maps(q, k, v, wq, bq, wk, bk, wv, bv, wo)
    res = run_bass_kernel_spmd(nc, in_maps, list(range(NCORES)))

    out = np.empty((B, S, D), np.float32)
    for b in range(B):
        acc = res.results[TP * b]["outT"].astype(np.float32)
        for t in range(1, TP):
            acc = acc + res.results[TP * b + t]["outT"].astype(np.float32)
        out[b] = acc.T + bo[None, :]
    return out
